# revision 3
# baseline (speedup 1.0000x reference)
"""DLinear forecast model as a single fused matmul on 8 TRN2 NeuronCores.

The model is out[b,p,c] = relu( sum_t seasonal[b,t,c]*Ws[p,t] + bs[p]
                               + sum_t trend[b,t,c]*Wt[p,t]    + bt[p] )
with trend = moving_avg(x) (kernel 5, edge pad) and seasonal = x - trend.
The moving average is a linear map over the time axis: trend = A @ x with
A [336,336].  Folding it into the weights gives a single matmul:
    out = relu(W_eff @ x[b] + bias),  W_eff = Ws + (Wt - Ws) @ A
Sharding: data-parallel over batch (64 = 8 cores x 8).
"""

import numpy as np
import ml_dtypes

import concourse.bass as bass
import concourse.mybir as mybir
from concourse.tile import TileContext
from concourse.bass_utils import run_bass_kernel_spmd

# Problem shapes (hardcoded per contract)
B, T, C = 64, 336, 1782
P_OUT = 720
N_CORES = 8
B_LOC = B // N_CORES  # 8 batches per core

KCH = 112   # contraction chunk (3 chunks of 112 = 336)
NK = 3
MCH = 120   # output-partition chunk (6 chunks of 120 = 720)
NM = 6
NCH = 512   # free-dim chunk (512,512,512,246)
N_SPLITS = [(i * NCH, min(NCH, C - i * NCH)) for i in range((C + NCH - 1) // NCH)]

BF16 = mybir.dt.bfloat16
F32 = mybir.dt.float32


def _split_excess_waits(nc, limit=1):
    """walrus in this toolchain rejects >limit sem-waits per instruction; move
    the extras onto injected same-engine NoOps immediately before it (same
    engine queue => program order => semantics preserved)."""
    seq = 0
    for f in nc.m.functions:
        for bb in f.blocks:
            new = []
            for inst in bb.instructions:
                si = inst.sync_info
                if si is not None and si.on_wait and len(si.on_wait) > limit:
                    waits = list(si.on_wait)
                    head, tail = waits[:-limit], waits[-limit:]
                    for w in head:
                        seq += 1
                        nop = mybir.InstNoOp(
                            name=f"{inst.name}-prewait{seq}", engine=inst.engine
                        )
                        nop.sync_info = mybir.SyncInfo(on_wait=[w], on_update=[])
                        new.append(nop)
                    inst.sync_info = mybir.SyncInfo(on_wait=tail, on_update=si.on_update)
                new.append(inst)
            bb.instructions = new


def build_kernel():
    nc = bass.Bass()
    x = nc.declare_dram_parameter("x", [B_LOC, T, C], BF16, isOutput=False)
    w = nc.declare_dram_parameter("w", [T, P_OUT], BF16, isOutput=False)
    bias = nc.declare_dram_parameter("bias", [MCH, NM], F32, isOutput=False)
    out = nc.declare_dram_parameter("out", [B_LOC, P_OUT, C], BF16, isOutput=True)

    with TileContext(nc) as tc:
        with (
            tc.tile_pool(name="wpool", bufs=1) as wpool,
            tc.tile_pool(name="bpool", bufs=1) as bpool,
            tc.tile_pool(name="xpool", bufs=2) as xpool,
            tc.tile_pool(name="opool", bufs=3) as opool,
            tc.tile_pool(name="psum", bufs=8, space="PSUM") as pspool,
        ):
            wt = []
            for k in range(NK):
                t = wpool.tile([KCH, P_OUT], BF16, tag=f"w{k}")
                nc.sync.dma_start(out=t[:], in_=w[k * KCH : (k + 1) * KCH, :])
                wt.append(t)
            bt = bpool.tile([MCH, NM], F32)
            nc.sync.dma_start(out=bt[:], in_=bias[:])

            for b in range(B_LOC):
                xt = []
                for k in range(NK):
                    t = xpool.tile([KCH, C], BF16, tag=f"x{k}")
                    nc.sync.dma_start(
                        out=t[:], in_=x[b, k * KCH : (k + 1) * KCH, :]
                    )
                    xt.append(t)
                for m in range(NM):
                    ot = opool.tile([MCH, C], BF16, tag="o")
                    for ni, (noff, nw) in enumerate(N_SPLITS):
                        ps = pspool.tile([MCH, NCH], F32, tag="ps")
                        for k in range(NK):
                            nc.tensor.matmul(
                                ps[:, :nw],
                                wt[k][:, m * MCH : (m + 1) * MCH],
                                xt[k][:, noff : noff + nw],
                                start=(k == 0),
                                stop=(k == NK - 1),
                            )
                        if ni % 2 == 0:
                            nc.scalar.activation(
                                ot[:, noff : noff + nw],
                                ps[:, :nw],
                                mybir.ActivationFunctionType.Relu,
                                bias=bt[:, m : m + 1],
                            )
                        else:
                            nc.vector.tensor_scalar(
                                ot[:, noff : noff + nw],
                                ps[:, :nw],
                                bt[:, m : m + 1],
                                0.0,
                                op0=mybir.AluOpType.add,
                                op1=mybir.AluOpType.max,
                            )
                    nc.sync.dma_start(
                        out=out[b, m * MCH : (m + 1) * MCH, :], in_=ot[:]
                    )

    _split_excess_waits(nc)
    return nc


def host_weights(W_seasonal, b_seasonal, W_trend, b_trend):
    """Fold the moving average into one weight matrix (f64 precision)."""
    K, PAD = 5, 2
    A = np.zeros((T, T), dtype=np.float64)
    idx = np.arange(T)
    for d in range(-PAD, PAD + 1):
        np.add.at(A, (idx, np.clip(idx + d, 0, T - 1)), 1.0 / K)
    Ws = W_seasonal.astype(np.float64)
    Wt = W_trend.astype(np.float64)
    W_eff = Ws + (Wt - Ws) @ A  # [720, 336]
    bias = (b_seasonal.astype(np.float64) + b_trend.astype(np.float64)).astype(
        np.float32
    )
    wT = np.ascontiguousarray(W_eff.T.astype(np.float32)).astype(ml_dtypes.bfloat16)
    bias_tiled = np.ascontiguousarray(bias.reshape(NM, MCH).T)  # [120, 6]
    return wT, bias_tiled


def make_in_maps(x, W_seasonal, b_seasonal, W_trend, b_trend):
    wT, bias_tiled = host_weights(W_seasonal, b_seasonal, W_trend, b_trend)
    xb = np.asarray(x).astype(ml_dtypes.bfloat16)
    return [
        {
            "x": np.ascontiguousarray(xb[i * B_LOC : (i + 1) * B_LOC]),
            "w": wT,
            "bias": bias_tiled,
        }
        for i in range(N_CORES)
    ]


def kernel(x, W_seasonal, b_seasonal, W_trend, b_trend):
    in_maps = make_in_maps(x, W_seasonal, b_seasonal, W_trend, b_trend)
    nc = build_kernel()
    res = run_bass_kernel_spmd(nc, in_maps, core_ids=list(range(N_CORES)))
    parts = [res.results[i]["out"].astype(np.float32) for i in range(N_CORES)]
    return np.concatenate(parts, axis=0)


# revision 6
# speedup vs baseline: 3.5238x; 3.5238x over previous
"""DLinear forecast model as a single fused matmul on 8 TRN2 NeuronCores.

The model is out[b,p,c] = relu( sum_t seasonal[b,t,c]*Ws[p,t] + bs[p]
                               + sum_t trend[b,t,c]*Wt[p,t]    + bt[p] )
with trend = moving_avg(x) (kernel 5, edge pad) and seasonal = x - trend.
The moving average is a linear map over the time axis: trend = A @ x with
A [336,336].  Folding it into the weights gives a single matmul:
    out = relu(W_eff @ x[b] + bias),  W_eff = Ws + (Wt - Ws) @ A
Sharding: data-parallel over batch (64 = 8 cores x 8).
"""

import numpy as np
import ml_dtypes

import concourse.bass as bass
import concourse.mybir as mybir
from concourse.tile import TileContext
from concourse.bass_utils import run_bass_kernel_spmd

# Problem shapes (hardcoded per contract)
B, T, C = 64, 336, 1782
P_OUT = 720
N_CORES = 8
B_LOC = B // N_CORES  # 8 batches per core

KCH = 112   # contraction chunk (3 chunks of 112 = 336)
NK = 3
MCH = 120   # output-partition chunk (6 chunks of 120 = 720)
NM = 6
NCH = 512   # free-dim chunk (512,512,512,246)
N_SPLITS = [(i * NCH, min(NCH, C - i * NCH)) for i in range((C + NCH - 1) // NCH)]

BF16 = mybir.dt.bfloat16
F32 = mybir.dt.float32


def _split_excess_waits(nc, limit=1):
    """walrus in this toolchain rejects >limit sem-waits per instruction; move
    the extras onto injected same-engine NoOps immediately before it (same
    engine queue => program order => semantics preserved)."""
    seq = 0
    for f in nc.m.functions:
        for bb in f.blocks:
            new = []
            for inst in bb.instructions:
                si = inst.sync_info
                if si is not None and si.on_wait and len(si.on_wait) > limit:
                    waits = list(si.on_wait)
                    head, tail = waits[:-limit], waits[-limit:]
                    for w in head:
                        seq += 1
                        nop = mybir.InstNoOp(
                            name=f"{inst.name}-prewait{seq}", engine=inst.engine
                        )
                        nop.sync_info = mybir.SyncInfo(on_wait=[w], on_update=[])
                        new.append(nop)
                    inst.sync_info = mybir.SyncInfo(on_wait=tail, on_update=si.on_update)
                new.append(inst)
            bb.instructions = new


def build_kernel(reps=1):
    nc = bass.Bass()
    x = nc.declare_dram_parameter("x", [B_LOC, T, C], BF16, isOutput=False)
    w = nc.declare_dram_parameter("w", [T, P_OUT], BF16, isOutput=False)
    bias = nc.declare_dram_parameter("bias", [MCH, NM], F32, isOutput=False)
    out = nc.declare_dram_parameter("out", [B_LOC, P_OUT, C], BF16, isOutput=True)

    with TileContext(nc) as tc:
        with (
            tc.tile_pool(name="wpool", bufs=1) as wpool,
            tc.tile_pool(name="bpool", bufs=1) as bpool,
            tc.tile_pool(name="xpool", bufs=3) as xpool,
            tc.tile_pool(name="opool", bufs=4) as opool,
            tc.tile_pool(name="psum", bufs=2, space="PSUM") as pspool,
        ):
            wt = []
            for k in range(NK):
                t = wpool.tile([KCH, P_OUT], BF16, tag=f"w{k}")
                nc.sync.dma_start(out=t[:], in_=w[k * KCH : (k + 1) * KCH, :])
                wt.append(t)
            bt = bpool.tile([MCH, NM], F32)
            nc.sync.dma_start(out=bt[:], in_=bias[:])

            for b in range(B_LOC * reps):
                b = b % B_LOC
                xt = []
                for k in range(NK):
                    t = xpool.tile([KCH, C], BF16, tag=f"x{k}")
                    nc.sync.dma_start(
                        out=t[:], in_=x[b, k * KCH : (k + 1) * KCH, :]
                    )
                    xt.append(t)
                for m in range(NM):
                    ot = opool.tile([MCH, C], BF16, tag="o")
                    ps = pspool.tile([MCH, 4 * NCH], F32, tag="ps")
                    for noff, nw in N_SPLITS:
                        for k in range(NK):
                            nc.tensor.matmul(
                                ps[:, noff : noff + nw],
                                wt[k][:, m * MCH : (m + 1) * MCH],
                                xt[k][:, noff : noff + nw],
                                start=(k == 0),
                                stop=(k == NK - 1),
                            )
                    # bias + relu + bf16 cast, split across ScalarE / VectorE
                    half = C // 2  # 891
                    nc.scalar.activation(
                        ot[:, :half],
                        ps[:, :half],
                        mybir.ActivationFunctionType.Relu,
                        bias=bt[:, m : m + 1],
                    )
                    nc.vector.tensor_scalar(
                        ot[:, half:C],
                        ps[:, half:C],
                        bt[:, m : m + 1],
                        0.0,
                        op0=mybir.AluOpType.add,
                        op1=mybir.AluOpType.max,
                    )
                    nc.sync.dma_start(
                        out=out[b, m * MCH : (m + 1) * MCH, :], in_=ot[:]
                    )

    _split_excess_waits(nc)
    return nc


def host_weights(W_seasonal, b_seasonal, W_trend, b_trend):
    """Fold the moving average into one weight matrix (f64 precision)."""
    K, PAD = 5, 2
    A = np.zeros((T, T), dtype=np.float64)
    idx = np.arange(T)
    for d in range(-PAD, PAD + 1):
        np.add.at(A, (idx, np.clip(idx + d, 0, T - 1)), 1.0 / K)
    Ws = W_seasonal.astype(np.float64)
    Wt = W_trend.astype(np.float64)
    W_eff = Ws + (Wt - Ws) @ A  # [720, 336]
    bias = (b_seasonal.astype(np.float64) + b_trend.astype(np.float64)).astype(
        np.float32
    )
    wT = np.ascontiguousarray(W_eff.T.astype(np.float32)).astype(ml_dtypes.bfloat16)
    bias_tiled = np.ascontiguousarray(bias.reshape(NM, MCH).T)  # [120, 6]
    return wT, bias_tiled


def make_in_maps(x, W_seasonal, b_seasonal, W_trend, b_trend):
    wT, bias_tiled = host_weights(W_seasonal, b_seasonal, W_trend, b_trend)
    xb = np.asarray(x).astype(ml_dtypes.bfloat16)
    return [
        {
            "x": np.ascontiguousarray(xb[i * B_LOC : (i + 1) * B_LOC]),
            "w": wT,
            "bias": bias_tiled,
        }
        for i in range(N_CORES)
    ]


def kernel(x, W_seasonal, b_seasonal, W_trend, b_trend):
    in_maps = make_in_maps(x, W_seasonal, b_seasonal, W_trend, b_trend)
    nc = build_kernel()
    res = run_bass_kernel_spmd(nc, in_maps, core_ids=list(range(N_CORES)))
    parts = [res.results[i]["out"].astype(np.float32) for i in range(N_CORES)]
    return np.concatenate(parts, axis=0)


# revision 9
# speedup vs baseline: 3.7552x; 1.0657x over previous
"""DLinear forecast model as a single fused matmul on 8 TRN2 NeuronCores.

The model is out[b,p,c] = relu( sum_t seasonal[b,t,c]*Ws[p,t] + bs[p]
                               + sum_t trend[b,t,c]*Wt[p,t]    + bt[p] )
with trend = moving_avg(x) (kernel 5, edge pad) and seasonal = x - trend.
The moving average is a linear map over the time axis: trend = A @ x with
A [336,336].  Folding it into the weights gives a single matmul:
    out = relu(W_eff @ x[b] + bias),  W_eff = Ws + (Wt - Ws) @ A
Sharding: data-parallel over batch (64 = 8 cores x 8).
"""

import numpy as np
import ml_dtypes

import concourse.bass as bass
import concourse.mybir as mybir
from concourse.tile import TileContext
from concourse.bass_utils import run_bass_kernel_spmd

# Problem shapes (hardcoded per contract)
B, T, C = 64, 336, 1782
P_OUT = 720
N_CORES = 8
B_LOC = B // N_CORES  # 8 batches per core

KCH = 112   # contraction chunk (3 chunks of 112 = 336)
NK = 3
MCH = 120   # output-partition chunk (6 chunks of 120 = 720)
NM = 6
NCH = 512   # free-dim chunk (512,512,512,246)
N_SPLITS = [(i * NCH, min(NCH, C - i * NCH)) for i in range((C + NCH - 1) // NCH)]

BF16 = mybir.dt.bfloat16
F32 = mybir.dt.float32


def _split_excess_waits(nc, limit=1):
    """walrus in this toolchain rejects >limit sem-waits per instruction; move
    the extras onto injected same-engine NoOps immediately before it (same
    engine queue => program order => semantics preserved)."""
    seq = 0
    for f in nc.m.functions:
        for bb in f.blocks:
            new = []
            for inst in bb.instructions:
                si = inst.sync_info
                if si is not None and si.on_wait and len(si.on_wait) > limit:
                    waits = list(si.on_wait)
                    head, tail = waits[:-limit], waits[-limit:]
                    for w in head:
                        seq += 1
                        nop = mybir.InstNoOp(
                            name=f"{inst.name}-prewait{seq}", engine=inst.engine
                        )
                        nop.sync_info = mybir.SyncInfo(on_wait=[w], on_update=[])
                        new.append(nop)
                    inst.sync_info = mybir.SyncInfo(on_wait=tail, on_update=si.on_update)
                new.append(inst)
            bb.instructions = new


def build_kernel(reps=1):
    nc = bass.Bass()
    x = nc.declare_dram_parameter("x", [B_LOC, T, C], BF16, isOutput=False)
    w = nc.declare_dram_parameter("w", [T, P_OUT], BF16, isOutput=False)
    bias = nc.declare_dram_parameter("bias", [MCH, NM], F32, isOutput=False)
    out = nc.declare_dram_parameter("out", [B_LOC, P_OUT, C], BF16, isOutput=True)

    with TileContext(nc) as tc:
        with (
            tc.tile_pool(name="wpool", bufs=1) as wpool,
            tc.tile_pool(name="bpool", bufs=1) as bpool,
            tc.tile_pool(name="xpool", bufs=3) as xpool,
            tc.tile_pool(name="opool", bufs=4) as opool,
            tc.tile_pool(name="psum", bufs=2, space="PSUM") as pspool,
        ):
            wt = []
            for k in range(NK):
                t = wpool.tile([KCH, P_OUT], BF16, tag=f"w{k}")
                nc.sync.dma_start(out=t[:], in_=w[k * KCH : (k + 1) * KCH, :])
                wt.append(t)
            bt = bpool.tile([MCH, NM], F32)
            nc.sync.dma_start(out=bt[:], in_=bias[:])

            for b in range(B_LOC * reps):
                b = b % B_LOC
                xt = []
                for k in range(NK):
                    t = xpool.tile([KCH, C], BF16, tag=f"x{k}")
                    nc.sync.dma_start(
                        out=t[:], in_=x[b, k * KCH : (k + 1) * KCH, :]
                    )
                    xt.append(t)
                for m in range(NM):
                    ot = opool.tile([MCH, C], BF16, tag="o")
                    ps = pspool.tile([MCH, 4 * NCH], F32, tag="ps")
                    for noff, nw in N_SPLITS:
                        for k in range(NK):
                            nc.tensor.matmul(
                                ps[:, noff : noff + nw],
                                wt[k][:, m * MCH : (m + 1) * MCH],
                                xt[k][:, noff : noff + nw],
                                start=(k == 0),
                                stop=(k == NK - 1),
                            )
                    # bias + relu + bf16 cast, split across ScalarE / VectorE
                    half = C // 2  # 891
                    nc.scalar.activation(
                        ot[:, :half],
                        ps[:, :half],
                        mybir.ActivationFunctionType.Relu,
                        bias=bt[:, m : m + 1],
                    )
                    nc.vector.tensor_scalar(
                        ot[:, half:C],
                        ps[:, half:C],
                        bt[:, m : m + 1],
                        0.0,
                        op0=mybir.AluOpType.add,
                        op1=mybir.AluOpType.max,
                    )
                    nc.sync.dma_start(
                        out=out[b, m * MCH : (m + 1) * MCH, :], in_=ot[:]
                    )

    _split_excess_waits(nc)
    return nc


def host_weights(W_seasonal, b_seasonal, W_trend, b_trend):
    """Fold the moving average into one weight matrix (f64 precision)."""
    K, PAD = 5, 2
    A = np.zeros((T, T), dtype=np.float64)
    idx = np.arange(T)
    for d in range(-PAD, PAD + 1):
        np.add.at(A, (idx, np.clip(idx + d, 0, T - 1)), 1.0 / K)
    Ws = W_seasonal.astype(np.float64)
    Wt = W_trend.astype(np.float64)
    W_eff = Ws + (Wt - Ws) @ A  # [720, 336]
    bias = (b_seasonal.astype(np.float64) + b_trend.astype(np.float64)).astype(
        np.float32
    )
    wT = np.ascontiguousarray(W_eff.T.astype(np.float32)).astype(ml_dtypes.bfloat16)
    bias_tiled = np.ascontiguousarray(bias.reshape(NM, MCH).T)  # [120, 6]
    return wT, bias_tiled


def make_in_maps(x, W_seasonal, b_seasonal, W_trend, b_trend):
    wT, bias_tiled = host_weights(W_seasonal, b_seasonal, W_trend, b_trend)
    xb = np.asarray(x).astype(ml_dtypes.bfloat16)
    return [
        {
            "x": np.ascontiguousarray(xb[i * B_LOC : (i + 1) * B_LOC]),
            "w": wT,
            "bias": bias_tiled,
        }
        for i in range(N_CORES)
    ]


def kernel(x, W_seasonal, b_seasonal, W_trend, b_trend):
    x = np.asarray(x)
    W_seasonal = np.asarray(W_seasonal)
    b_seasonal = np.asarray(b_seasonal)
    W_trend = np.asarray(W_trend)
    b_trend = np.asarray(b_trend)
    in_maps = make_in_maps(x, W_seasonal, b_seasonal, W_trend, b_trend)
    for attempt in range(3):
        try:
            nc = build_kernel()
            res = run_bass_kernel_spmd(nc, in_maps, core_ids=list(range(N_CORES)))
            break
        except Exception:  # transient device wedge (NRT_EXEC_UNIT_...)
            if attempt == 2:
                raise
            import time as _time

            _time.sleep(20)
    parts = [res.results[i]["out"].astype(np.float32) for i in range(N_CORES)]
    return np.concatenate(parts, axis=0)
